# revision 1
# baseline (speedup 1.0000x reference)
"""Bass/Trainium2 kernel for a single-head causal decoder attention head.

Reference computation (fp32):
    k = x @ Wk; q = x @ Wq; v = x @ Wv            # [B,T,H]
    att = softmax(causal(q k^T / sqrt(H)))        # [B,T,T]
    out = att @ v                                 # [B,T,H]
with B=4, T=4096, C=1024, H=128.

Sharding: 8 cores = 4 batches x 2 query-interleave lanes (j in {0,1}).
Core (b, j) handles q-groups [(2i+j)*512, (2i+j+1)*512) for i in 0..3 and
runs a *uniform* kv-span schedule {1024, 2048, 3072, 4096} for groups
0..3, so all 8 cores execute the same instruction stream (SPMD, one NEFF)
while per-core DRAM data (x^T slices, q-column gather, mask stack) makes
the math come out right.  Causality beyond each group's true span is
enforced by additive -30000 masks on the last 8 kv chunks of each group.

Dataflow (per core, transposed land so no on-chip transposes are needed):
    KT [H, T]   = Wk^T x^T        (8 c-chunk matmuls per 512 kv cols)
    QT [H, 2048]= Wq^T xq^T
    V  [kv, H]  (32 blocks)       (lhsT = x^T chunk, rhs = Wv chunk)
    per q-group, per kv chunk c:
        S^T  = KT_c^T QT_g                 (PSUM [128kv, 512q])
        S^T += mask (last 8 chunks, DVE)
        P^T  = exp(S^T / sqrt(H))          (ACT, bf16 -> SBUF)
        outT += V_c^T P^T                  (PSUM [128H, 512q], accumulated)
        sums += ones^T P^T                 (PSUM [1, 512q], accumulated)
    outT / sums -> DRAM  (reciprocal + partition-broadcast + multiply)
"""

import sys

sys.path.insert(0, "/opt/trn_rl_repo")

import numpy as np
import ml_dtypes

import concourse.bass as bass
import concourse.mybir as mybir
import concourse.tile as tile
from concourse import bacc
from concourse.alu_op_type import AluOpType
from concourse.masks import make_identity
from concourse.bass_utils import run_bass_kernel_spmd

B, T, C, H = 4, 4096, 1024, 128
NCORES = 8
QG = 512                      # q-group width
NG = 4                        # q-groups per core
SPANS = [1024, 2048, 3072, 4096]  # uniform kv span per group index
CB = C // 128                 # 8 contraction chunks
TGRP = T // QG                # 8 kv col-groups for projections
SCALE = float(H) ** -0.5
MASKVAL = -30000.0

BF16 = mybir.dt.bfloat16
F32 = mybir.dt.float32
NPBF16 = ml_dtypes.bfloat16


def _build_program():
    nc = bacc.Bacc("TRN2", target_bir_lowering=False, debug=False)

    xt = nc.dram_tensor("xt", [C, T], BF16, kind="ExternalInput").ap()
    xtq = nc.dram_tensor("xtq", [C, NG * QG], BF16, kind="ExternalInput").ap()
    wk = nc.dram_tensor("wk", [C, H], BF16, kind="ExternalInput").ap()
    wq = nc.dram_tensor("wq", [C, H], BF16, kind="ExternalInput").ap()
    wv = nc.dram_tensor("wv", [C, H], BF16, kind="ExternalInput").ap()
    msk = nc.dram_tensor("msk", [8, 128, QG], BF16, kind="ExternalInput").ap()
    outT = nc.dram_tensor("outT", [H, NG * QG], F32, kind="ExternalOutput").ap()

    with tile.TileContext(nc) as tc:
        with (
            tc.tile_pool(name="const", bufs=1) as constp,
            tc.tile_pool(name="kvq", bufs=1) as kvqp,
            tc.tile_pool(name="xin", bufs=2) as xinp,
            tc.tile_pool(name="attb", bufs=4) as attp,
            tc.tile_pool(name="epi", bufs=2) as epip,
        ):
            # --- persistent SBUF tensors ---
            wks = constp.tile([128, CB * H], BF16, tag="wks")
            wqs = constp.tile([128, CB * H], BF16, tag="wqs")
            wvs = constp.tile([128, CB * H], BF16, tag="wvs")
            for eng, ws, w in (
                (nc.scalar, wks, wk), (nc.scalar, wqs, wq), (nc.gpsimd, wvs, wv)
            ):
                eng.dma_start(
                    ws.rearrange("p (c h) -> p c h", c=CB),
                    w.rearrange("(c p) h -> p c h", p=128),
                )
            masks = constp.tile([128, 8 * QG], BF16, tag="masks")
            ident = constp.tile([128, 128], BF16, tag="ident")
            make_identity(nc, ident)

            KT = kvqp.tile([128, T], BF16, tag="KT")
            VT = kvqp.tile([128, T], BF16, tag="VT")
            VV = kvqp.tile([128, (T // 128) * H], BF16, tag="VV")
            QT = kvqp.tile([128, NG * QG], BF16, tag="QT")
            ones = kvqp.tile([128, 128], BF16, tag="ones")
            nc.vector.memset(ones, 1.0)

            # --- phase 1: projections ---
            with tc.tile_pool(name="pp", bufs=2, space="PSUM") as ppool:
                xtr = xt.rearrange("(c p) t -> p c t", p=128)
                for tg in range(TGRP):
                    xg = xinp.tile([128, CB * QG], BF16, tag="xg", bufs=TGRP)
                    xgv = xg.rearrange("p (c q) -> p c q", c=CB)
                    if tg == 0:
                        nc.sync.dma_start(xgv[:, 0:1], xtr[:, 0:1, 0:QG])
                        nc.sync.dma_start(xgv[:, 1:CB], xtr[:, 1:CB, 0:QG])
                    else:
                        nc.sync.dma_start(
                            xgv, xtr[:, :, tg * QG:(tg + 1) * QG]
                        )
                    kps = ppool.tile([128, QG], F32, tag="kps")
                    for c in range(CB):
                        nc.tensor.matmul(
                            kps,
                            lhsT=wks[:, c * H:(c + 1) * H],
                            rhs=xg[:, c * QG:(c + 1) * QG],
                            start=(c == 0),
                            stop=(c == CB - 1),
                        )
                    nc.any.tensor_copy(KT[:, tg * QG:(tg + 1) * QG], kps)
                    vps = ppool.tile([128, QG], F32, tag="vps")
                    for c in range(CB):
                        nc.tensor.matmul(
                            vps,
                            lhsT=wvs[:, c * H:(c + 1) * H],
                            rhs=xg[:, c * QG:(c + 1) * QG],
                            start=(c == 0),
                            stop=(c == CB - 1),
                        )
                    nc.any.tensor_copy(VT[:, tg * QG:(tg + 1) * QG], vps)
                    for tb in range(QG // 128):
                        t = tg * (QG // 128) + tb
                        tps = ppool.tile([128, 128], BF16, tag="tps")
                        nc.tensor.transpose(
                            tps, VT[:, t * 128:(t + 1) * 128], ident
                        )
                        nc.vector.tensor_copy(VV[:, t * H:(t + 1) * H], tps)
                for i in range(NG):
                    xq = xinp.tile([128, CB * QG], BF16, tag="xq", bufs=NG)
                    nc.sync.dma_start(
                        xq.rearrange("p (c q) -> p c q", c=CB),
                        xtq.rearrange("(c p) t -> p c t", p=128)[:, :, i * QG:(i + 1) * QG],
                    )
                    qps = ppool.tile([128, QG], F32, tag="qps")
                    for c in range(CB):
                        nc.tensor.matmul(
                            qps,
                            lhsT=wqs[:, c * H:(c + 1) * H],
                            rhs=xq[:, c * QG:(c + 1) * QG],
                            start=(c == 0),
                            stop=(c == CB - 1),
                        )
                    nc.any.tensor_copy(QT[:, i * QG:(i + 1) * QG], qps)

            # --- phase 2: attention ---
            nc.sync.dma_start(
                masks.rearrange("p (m q) -> p m q", m=8),
                msk.rearrange("m p q -> p m q"),
            )
            with tc.tile_pool(name="ap", bufs=2, space="PSUM") as apool:
                for i in range(NG):
                    span = SPANS[i]
                    nchunks = span // 128
                    otps = apool.tile([128, QG], F32, tag="otps", bufs=1)
                    smps = apool.tile([128, QG], F32, tag="smps", bufs=1)
                    qg = QT[:, i * QG:(i + 1) * QG]
                    for cp in range(nchunks // 2):
                        c0 = 2 * cp
                        sps = apool.tile([128, 2 * QG], F32, tag="sps", bufs=3)
                        for h in range(2):
                            nc.tensor.matmul(
                                sps[:, h * QG:(h + 1) * QG],
                                lhsT=KT[:, (c0 + h) * 128:(c0 + h + 1) * 128],
                                rhs=qg,
                                start=True,
                                stop=True,
                            )
                        pt = attp.tile([128, 2 * QG], BF16, tag="pt")
                        nc.scalar.activation(
                            pt, sps, mybir.ActivationFunctionType.Exp, scale=SCALE
                        )
                        m = c0 - (nchunks - 8)
                        if m >= 0:
                            nc.vector.tensor_tensor(
                                pt, pt, masks[:, m * QG:(m + 2) * QG],
                                op=AluOpType.mult,
                            )
                        for h in range(2):
                            c = c0 + h
                            ph = pt[:, h * QG:(h + 1) * QG]
                            nc.tensor.matmul(
                                otps,
                                lhsT=VV[:, c * H:(c + 1) * H],
                                rhs=ph,
                                start=(c == 0),
                                stop=(c == nchunks - 1),
                            )
                            nc.tensor.matmul(
                                smps,
                                lhsT=ones,
                                rhs=ph,
                                start=(c == 0),
                                stop=(c == nchunks - 1),
                            )
                    rb = epip.tile([128, QG], F32, tag="rb")
                    nc.vector.reciprocal_approx_fast(rb, smps)
                    ot = epip.tile([128, QG], F32, tag="ot")
                    nc.vector.tensor_tensor(ot, otps, rb, op=AluOpType.mult)
                    nc.sync.dma_start(outT[:, i * QG:(i + 1) * QG], ot)

    if not nc.is_finalized():
        nc.finalize()
    return nc


_NC_CACHE = None


def _get_program():
    global _NC_CACHE
    if _NC_CACHE is None:
        _NC_CACHE = _build_program()
    return _NC_CACHE


def _make_masks(j: int) -> np.ndarray:
    """Mask stack [8, 128, QG] for lane j (f32, 0 or MASKVAL).

    Slot s applies to kv chunk at offset K0 = g - (1024 - j*512) + 128*s
    relative ... concretely: for lane j, the last 8 chunks of each group's
    span get slots 0..7; masked iff global kv > global q, i.e.
    128*(s - 4 + (1 - j) * 4 ... reduces to: kv_i + 128*s - (4 - 4*j)*128 > q_j
    """
    out = np.zeros((8, 128, QG), NPBF16)
    kv = np.arange(128)[:, None]
    q = np.arange(QG)[None, :]
    for s in range(8):
        # lane j: slot s covers the chunk at K0 = g + 128*s - 512*j;
        # multiplicative mask: 0 where kv_global > q_global else 1
        rel = 128 * s - 512 * j
        out[s] = np.where(rel + kv > q, 0.0, 1.0).astype(NPBF16)
    return out


def _run(inputs: dict, trace: bool = False, trace_kwargs: dict | None = None):
    x = np.asarray(inputs["x"], np.float32)
    Wk = np.asarray(inputs["Wk"], np.float32)
    Wq = np.asarray(inputs["Wq"], np.float32)
    Wv = np.asarray(inputs["Wv"], np.float32)

    nc = _get_program()

    wk16 = Wk.astype(NPBF16)
    wq16 = Wq.astype(NPBF16)
    wv16 = Wv.astype(NPBF16)
    msks = [_make_masks(j) for j in range(2)]

    in_maps = []
    for b in range(B):
        xtb = np.ascontiguousarray(x[b].T).astype(NPBF16)  # [C, T]
        for j in range(2):
            xtq = np.concatenate(
                [xtb[:, (2 * i + j) * QG:(2 * i + j + 1) * QG] for i in range(NG)],
                axis=1,
            )
            in_maps.append(
                {
                    "xt": xtb,
                    "xtq": np.ascontiguousarray(xtq),
                    "wk": wk16,
                    "wq": wq16,
                    "wv": wv16,
                    "msk": msks[j],
                }
            )

    res = run_bass_kernel_spmd(
        nc,
        in_maps,
        core_ids=list(range(NCORES)),
        trace=trace,
        **(trace_kwargs or {}),
    )

    out = np.empty((B, T, H), np.float32)
    for core in range(NCORES):
        b, j = divmod(core, 2)
        oT = np.asarray(res.results[core]["outT"], np.float32)  # [H, NG*QG]
        for i in range(NG):
            g = (2 * i + j) * QG
            out[b, g:g + QG, :] = oT[:, i * QG:(i + 1) * QG].T
    return out, res


def kernel(**inputs) -> np.ndarray:
    out, _ = _run(inputs, trace=False)
    return out



# revision 8
# speedup vs baseline: 1.1919x; 1.1919x over previous
"""Bass/Trainium2 kernel for a single-head causal decoder attention head.

Reference (fp32):
    k = x @ Wk; q = x @ Wq; v = x @ Wv            # [B,T,H]
    att = softmax(causal(q k^T / sqrt(H)))        # [B,T,T]
    out = att @ v                                 # [B,T,H]
with B=4, T=4096, C=1024, H=128.

Sharding: 8 cores = 4 batches x 2 query-interleave lanes (j in {0,1}).
q-blocks are 256 wide (16 blocks); lane j owns blocks {2i+j}.  The host
permutes x^T columns per lane (lane 1 swaps adjacent 256-col blocks) so
that in *position* space every core's slot i has its q-block at position
2i and a causal kv window of positions [0, 2i+2) == kv cols [0, 512(i+1)).
All 8 cores run one uniform SPMD program; lane differences live entirely
in data (the column permutation and a [128, 4*256] mask tile).

Per-core dataflow (transposed land, kv on partitions):
    phase 1 (per 512-col tg w): KT chunk = Wk^T xg, V chunk directly in
    [kv,H] blocks (lhsT = xg cols), Q block = Wq^T xg[:, :256];
    V is stored as fp8e4 (VV8) + bf16 for kv<1024 (VVb).
    phase 2 (slot i, batches of 4 kv-chunks):
      S^T  = KT_c^T Q_i                  (PSUM [128kv, 4, 256q] f32)
      P^T  = exp(S^T/sqrt(H) - 3)        (ACT -> fp8 slots>=2, bf16 else)
      P^T *= mask                        (last batch only, DVE)
      out  += V_c^T P^T ; sums += 1^T P^T
        (fp8 DoubleRow pairs for slots>=2, bf16 for slots 0,1;
         out/sums share one PSUM bank: first PV starts the bank zero,
         everything else accumulates with start=False)
    out/sums -> DRAM via reciprocal + multiply.
Slot 7 is processed incrementally (batch w during step w+1) so the tail
after the last projection is a single batch.  Projection matmuls are
woven between attention batches to cover ACT exp latency.
"""

import sys

sys.path.insert(0, "/opt/trn_rl_repo")

import numpy as np
import ml_dtypes

import concourse.bass as bass
import concourse.mybir as mybir
import concourse.tile as tile
from concourse import bacc
from concourse.alu_op_type import AluOpType
from concourse.bass_utils import run_bass_kernel_spmd

B, T, C, H = 4, 4096, 1024, 128
NCORES = 8
QG = 256                      # q-block width
NSLOT = 8                     # slots (q-blocks) per core
CB = C // 128                 # 8 contraction chunks
TGW = 512                     # projection column-group width (2 positions)
NTG = T // TGW                # 8
SCALE = float(H) ** -0.5
EXPBIAS = -3.0

BF16 = mybir.dt.bfloat16
FP8 = mybir.dt.float8e4
F32 = mybir.dt.float32
NPBF16 = ml_dtypes.bfloat16
DR = mybir.MatmulPerfMode.DoubleRow
EXP = mybir.ActivationFunctionType.Exp


def _build_program():
    nc = bacc.Bacc("TRN2", target_bir_lowering=False, debug=False)

    xt = nc.dram_tensor("xt", [C, T], BF16, kind="ExternalInput").ap()
    wk = nc.dram_tensor("wk", [C, H], BF16, kind="ExternalInput").ap()
    wq = nc.dram_tensor("wq", [C, H], BF16, kind="ExternalInput").ap()
    wv = nc.dram_tensor("wv", [C, H], BF16, kind="ExternalInput").ap()
    msk = nc.dram_tensor("msk", [128, 4 * QG], BF16, kind="ExternalInput").ap()
    outT = nc.dram_tensor("outT", [H, NSLOT * QG], F32, kind="ExternalOutput").ap()

    xtr = xt.rearrange("(c p) t -> p c t", p=128)

    with tile.TileContext(nc) as tc:
        with (
            tc.tile_pool(name="const", bufs=1) as constp,
            tc.tile_pool(name="kvq", bufs=1) as kvqp,
            tc.tile_pool(name="xin", bufs=2) as xinp,
            tc.tile_pool(name="attb", bufs=3) as attp,
            tc.tile_pool(name="epi", bufs=2) as epip,
            tc.tile_pool(name="sp", bufs=2, space="PSUM") as spool,
            tc.tile_pool(name="cp", bufs=2, space="PSUM") as cpool,
            tc.tile_pool(name="op", bufs=1, space="PSUM") as opool,
        ):
            # ---- persistent SBUF ----
            wks = constp.tile([128, CB * H], BF16, tag="wks")
            wqs = constp.tile([128, CB * H], BF16, tag="wqs")
            wvs = constp.tile([128, CB * H], BF16, tag="wvs")
            nc.scalar.dma_start(
                wks.rearrange("p (c h) -> p c h", c=CB),
                wk.rearrange("(c p) h -> p c h", p=128),
            )
            nc.scalar.dma_start(
                wqs.rearrange("p (c h) -> p c h", c=CB),
                wq.rearrange("(c p) h -> p c h", p=128),
            )
            xq7 = constp.tile([128, CB * QG], BF16, tag="xq7")
            nc.scalar.dma_start(
                xq7.rearrange("p (c q) -> p c q", c=CB),
                xtr[:, :, 7 * TGW:7 * TGW + QG],
            )
            nc.scalar.dma_start(
                wvs.rearrange("p (c h) -> p c h", c=CB),
                wv.rearrange("(c p) h -> p c h", p=128),
            )
            masks = constp.tile([128, 4 * QG], BF16, tag="masks")
            nc.gpsimd.dma_start(masks, msk)

            onesb = constp.tile([128, H], BF16, tag="onesb")
            nc.vector.memset(onesb, 1.0)
            biast = constp.tile([128, 1], F32, tag="biast")
            nc.vector.memset(biast, EXPBIAS)
            ones8 = constp.tile([128, 2 * H], FP8, tag="ones8")
            nc.vector.memset(ones8, 1.0)
            ones8v = ones8.rearrange("p (k h) -> p k h", k=2)

            KT = kvqp.tile([128, T], BF16, tag="KT")
            QT = kvqp.tile([128, NSLOT * QG], BF16, tag="QT")
            VV8 = kvqp.tile([128, (T // 128) * H], FP8, tag="VV8")
            VV8v = VV8.rearrange("p (k h) -> p k h", k=T // 128)
            VVb = kvqp.tile([128, 8 * H], BF16, tag="VVb")
            VVbv = VVb.rearrange("p (k h) -> p k h", k=8)

            # os7: slot-7 out/sums accumulator, lives for the whole kernel.
            os7 = opool.tile([128, 2 * QG], F32, tag="os7")

            # ---------- emission helpers ----------
            def s_batch(i, b, fillers):
                """Emit S matmuls + exp (+mask) for (slot i, batch b).
                Returns (pt, fp8) for the later PV/sums emission."""
                fp8 = i >= 2
                sps = spool.tile([128, 4 * QG], F32, tag="sps")
                qg = QT[:, i * QG:(i + 1) * QG]
                for kb in range(4):
                    ch = 4 * b + kb
                    nc.tensor.matmul(
                        sps[:, kb * QG:(kb + 1) * QG],
                        lhsT=KT[:, ch * 128:(ch + 1) * 128],
                        rhs=qg,
                        start=(kb % 2 == 0),
                        stop=True,
                        skip_group_check=True,
                    )
                    if fillers and kb % 2 == 1:
                        fillers.pop(0)()
                pt = attp.tile(
                    [128, 4 * QG], FP8 if fp8 else BF16,
                    tag="pt8" if fp8 else "ptb",
                    bufs=3 if fp8 else 2,
                )
                nc.scalar.activation(pt, sps, EXP, bias=biast, scale=SCALE)
                if b == i:
                    nc.vector.tensor_tensor(pt, pt, masks, op=AluOpType.mult)
                return pt

            def pv_batch(i, b, pt, os):
                """Emit PV + sums matmuls for (slot i, batch b) into os."""
                if i >= 2:
                    ptv = pt.rearrange("p (k q) -> p k q", k=4)
                    npairs = 2 * (i + 1)
                    for p in range(2):
                        pair = 2 * b + p
                        c0 = 4 * b + 2 * p
                        nc.tensor.matmul(
                            os[:, 0:QG],
                            lhsT=VV8v[:, c0:c0 + 2, :],
                            rhs=ptv[:, 2 * p:2 * p + 2, :],
                            start=(pair == 0),
                            stop=(pair == npairs - 1),
                            perf_mode=DR,
                            skip_group_check=True,
                        )
                    for p in range(2):
                        pair = 2 * b + p
                        nc.tensor.matmul(
                            os[:, QG:2 * QG],
                            lhsT=ones8v,
                            rhs=ptv[:, 2 * p:2 * p + 2, :],
                            start=False,
                            stop=(pair == npairs - 1),
                            perf_mode=DR,
                            skip_group_check=True,
                        )
                else:
                    nch = 4 * (i + 1)
                    for kb in range(4):
                        ch = 4 * b + kb
                        nc.tensor.matmul(
                            os[:, 0:QG],
                            lhsT=VVbv[:, ch, :],
                            rhs=pt[:, kb * QG:(kb + 1) * QG],
                            start=(ch == 0),
                            stop=(ch == nch - 1),
                            skip_group_check=True,
                        )
                    for kb in range(4):
                        ch = 4 * b + kb
                        nc.tensor.matmul(
                            os[:, QG:2 * QG],
                            lhsT=onesb,
                            rhs=pt[:, kb * QG:(kb + 1) * QG],
                            start=False,
                            stop=(ch == nch - 1),
                            skip_group_check=True,
                        )

            def epilogue(i, os):
                rb = epip.tile([128, QG], F32, tag="rb")
                nc.vector.reciprocal_approx_fast(rb, os[:, QG:2 * QG])
                ot = epip.tile([128, QG], F32, tag="ot")
                nc.vector.tensor_tensor(ot, os[:, 0:QG], rb, op=AluOpType.mult)
                nc.scalar.dma_start(outT[:, i * QG:(i + 1) * QG], ot)

            # ---------- main steps ----------
            osc = None
            for w in range(NTG):
                # stream in this step's x columns
                xg = xinp.tile([128, CB * TGW], BF16, tag="xg")
                xgv = xg.rearrange("p (c q) -> p c q", c=CB)
                if w == 0:
                    for c0 in range(0, CB, 2):
                        nc.sync.dma_start(
                            xgv[:, c0:c0 + 2], xtr[:, c0:c0 + 2, 0:TGW]
                        )
                else:
                    nc.sync.dma_start(
                        xgv, xtr[:, :, w * TGW:(w + 1) * TGW]
                    )

                # K projection (always first: attention S needs fresh KT)
                kps = cpool.tile([128, TGW], F32, tag="pps")
                for c in range(CB):
                    nc.tensor.matmul(
                        kps,
                        lhsT=wks[:, c * H:(c + 1) * H],
                        rhs=xgv[:, c, :],
                        start=(c == 0),
                        stop=(c == CB - 1),
                    )
                nc.vector.tensor_copy(KT[:, w * TGW:(w + 1) * TGW], kps)

                if w == 0:
                    # slot-7 Q block, prefetched so slot 7 can run incrementally
                    q7 = cpool.tile([128, TGW], F32, tag="pps")
                    xq7v = xq7.rearrange("p (c q) -> p c q", c=CB)
                    for c in range(CB):
                        nc.tensor.matmul(
                            q7[:, 0:QG],
                            lhsT=wqs[:, c * H:(c + 1) * H],
                            rhs=xq7v[:, c, :],
                            start=(c == 0),
                            stop=(c == CB - 1),
                        )
                    nc.vector.tensor_copy(QT[:, 7 * QG:8 * QG], q7[:, 0:QG])

                # build the filler list: V projection (+ Q projection) pieces
                fillers = []
                vps = cpool.tile([128, TGW], F32, tag="pps")

                def mk_v(kb, w=w, vps=vps, xgv=xgv):
                    def emit():
                        for c in range(CB):
                            nc.tensor.matmul(
                                vps[:, kb * H:(kb + 1) * H],
                                lhsT=xgv[:, c, kb * 128:(kb + 1) * 128],
                                rhs=wvs[:, c * H:(c + 1) * H],
                                start=(kb == 0 and c == 0),
                                stop=(c == CB - 1),
                                skip_group_check=True,
                            )
                    return emit

                for kb in range(4):
                    fillers.append(mk_v(kb))

                def v_copy(w=w, vps=vps):
                    nc.vector.tensor_copy(
                        VV8v[:, 4 * w:4 * w + 4, :],
                        vps.rearrange("p (k h) -> p k h", k=4),
                    )
                    if w < 2:
                        nc.vector.tensor_copy(
                            VVbv[:, 4 * w:4 * w + 4, :],
                            vps.rearrange("p (k h) -> p k h", k=4),
                        )
                fillers.append(v_copy)

                if w < NTG - 1:
                    qps = cpool.tile([128, TGW], F32, tag="pps")

                    def mk_q(c0, w=w, qps=qps, xgv=xgv):
                        def emit():
                            for c in range(c0, c0 + 4):
                                nc.tensor.matmul(
                                    qps[:, 0:QG],
                                    lhsT=wqs[:, c * H:(c + 1) * H],
                                    rhs=xgv[:, c, 0:QG],
                                    start=(c == 0),
                                    stop=(c == CB - 1),
                                )
                        return emit
                    fillers.append(mk_q(0))
                    fillers.append(mk_q(4))

                    def q_copy(w=w, qps=qps):
                        nc.vector.tensor_copy(
                            QT[:, w * QG:(w + 1) * QG], qps[:, 0:QG]
                        )
                    fillers.append(q_copy)

                # attention work for this step: slot-7 batch (w-1), then all
                # batches of slot w-1, with fillers woven in.
                pend = []  # (i, b, pt, os) waiting for PV emission
                if w >= 1:
                    pend.append((7, w - 1, s_batch(7, w - 1, fillers), os7))
                if w >= 1:
                    i = w - 1
                    osc = opool.tile([128, 2 * QG], F32, tag="osc")
                    for b in range(i + 1):
                        if fillers:
                            fillers.pop(0)()
                        prev = pend.pop(0) if pend else None
                        pend.append((i, b, s_batch(i, b, fillers), osc))
                        if prev is not None:
                            pv_batch(prev[0], prev[1], prev[2], prev[3])
                while fillers:
                    fillers.pop(0)()
                for (pi, pb, ppt, pos) in pend:
                    pv_batch(pi, pb, ppt, pos)
                if w >= 1:
                    epilogue(w - 1, osc)

            # tail: slot 7's final batch + epilogue
            pt = s_batch(7, 7, [])
            pv_batch(7, 7, pt, os7)
            epilogue(7, os7)

    if not nc.is_finalized():
        nc.finalize()
    return nc


_NC_CACHE = None


def _get_program():
    global _NC_CACHE
    if _NC_CACHE is None:
        _NC_CACHE = _build_program()
    return _NC_CACHE


def _make_mask(j: int) -> np.ndarray:
    """[128, 4, QG] multiplicative mask for the last 4 kv-chunks of a slot.

    Chunks 0-1: the slot's own (diagonal) q-block vs kv positions 0..255:
    keep iff kv_within <= q_within (identical for both lanes).
    Chunks 2-3: the partner block at position 2i+1: lane 0's partner is the
    *future* block (all masked), lane 1's is the *past* block (all kept).
    """
    m = np.zeros((128, 4, QG), np.float32)
    u = np.arange(128)[:, None]
    v = np.arange(QG)[None, :]
    for cc in range(2):
        m[:, cc, :] = (128 * cc + u <= v).astype(np.float32)
    m[:, 2:4, :] = float(j)
    return m.reshape(128, 4 * QG).astype(NPBF16)


def _run(inputs: dict, trace: bool = False, trace_kwargs: dict | None = None):
    x = np.asarray(inputs["x"], np.float32)
    Wk = np.asarray(inputs["Wk"], np.float32)
    Wq = np.asarray(inputs["Wq"], np.float32)
    Wv = np.asarray(inputs["Wv"], np.float32)

    nc = _get_program()

    wk16 = Wk.astype(NPBF16)
    wq16 = Wq.astype(NPBF16)
    wv16 = Wv.astype(NPBF16)
    msks = [_make_mask(j) for j in range(2)]

    in_maps = []
    for b in range(B):
        xtb = np.ascontiguousarray(x[b].T).astype(NPBF16)  # [C, T]
        # lane 1 swaps adjacent 256-col blocks so its q-blocks sit at even
        # positions; lane 0 is the identity permutation.
        xsw = np.ascontiguousarray(
            xtb.reshape(C, NSLOT, 2, QG)[:, :, ::-1, :].reshape(C, T)
        )
        for j in range(2):
            in_maps.append(
                {
                    "xt": xtb if j == 0 else xsw,
                    "wk": wk16,
                    "wq": wq16,
                    "wv": wv16,
                    "msk": msks[j],
                }
            )

    res = run_bass_kernel_spmd(
        nc,
        in_maps,
        core_ids=list(range(NCORES)),
        trace=trace,
        **(trace_kwargs or {}),
    )

    out = np.empty((B, T, H), np.float32)
    for core in range(NCORES):
        b, j = divmod(core, 2)
        oT = np.asarray(res.results[core]["outT"], np.float32)  # [H, 8*QG]
        for i in range(NSLOT):
            g = (2 * i + j) * QG
            out[b, g:g + QG, :] = oT[:, i * QG:(i + 1) * QG].T
    return out, res


def kernel(**inputs) -> np.ndarray:
    out, _ = _run(inputs, trace=False)
    return out
